# revision 29
# baseline (speedup 1.0000x reference)
"""Trainium2 Bass kernel for nn_BiLSTM_57440892617018.

2-layer bidirectional LSTM (independent fw / bw stacks, merge_mode='ave'),
B=2048, T=200, D=U=128. Data-parallel over batch across 8 NeuronCores.

Per-core structure: ONE software-pipelined loop, 2 recurrent streams per
superstep (even = fw pair L0f/L1f with L1 lagging one timestep, odd = bw
pair on the time-reversed sequence). The schedule is built around keeping
the ACT engine (the busy-time bottleneck: 5 activations x 256 batch cols
per stream-timestep at ~0.83ns/col) 100% busy:

  ACT order per superstep s: [tanh_c(s-1) | sig_if(s) | tanh_g(s) | sig_o(s)]

tanh_c of the previous superstep runs while THIS superstep's DVE combine
(c = f*c + i*g) finishes, so ACT never waits on DVE. All DVE elementwise
work is emitted as scalar_tensor_tensor (4x DVE mode for fp16 SBUF
operands). x is pre-transposed on the host to [D, T, B] fp16 so DMA loads
land directly as matmul-ready [d, b] tiles (no PE transposes, no DVE
copies). PE runs fp16 matmuls: per superstep 16 gate matmuls (x/W first,
then h/U, g and o last for PSUM WAR friendliness) plus the layer-1 output
merge (0.5*I matmuls, which also produce the [b, d] layout the output DMA
needs). The PSUM->SBUF output copy runs on the otherwise-idle GPSIMD
engine.
"""

import numpy as np
import ml_dtypes

import concourse.bass as bass
import concourse.tile as tile
from concourse import bacc, mybir
from concourse.bass_utils import run_bass_kernel_spmd

F32 = mybir.dt.float32
DT2 = mybir.dt.float16
DT2_NP = np.float16
AF = mybir.ActivationFunctionType
ALU = mybir.AluOpType

B, T, D, U = 2048, 200, 128, 128
NCORES = 8
BL = B // NCORES          # 256 batch per core
NB = BL // 128            # 2 b-tiles
BCOL = 128 * NB           # 256 free columns (batch)

# gate order inside the reference 4u axis: i, f, g, o
GATE_COLS = {"i": 0, "f": 128, "g": 256, "o": 384}
SLOTS = ["Wi", "Wf", "Wo", "Wg", "Ui", "Uf", "Uo", "Ug"]
# W slot per gate / U slot per gate (indices into SLOTS)
WSLOT = {"i": 0, "f": 1, "g": 3, "o": 2}
USLOT = {"i": 4, "f": 5, "g": 7, "o": 6}

_cache = {}


def _wcol(di, l, slot):
    return ((di * 2 + l) * 8 + slot) * 128


def _build(uniform_bias, bias_val, Tn=T, reps=1, loop_R=0, tiny_x=False):
    nc = bacc.Bacc("TRN2", target_bir_lowering=False, debug=False,
                   num_devices=NCORES)
    Th = Tn // 2

    # x pre-transposed on host: [D, T, B] fp16 (contiguous b per (d, t))
    xs = nc.dram_tensor("xs", [D, 1 if tiny_x else Tn, BCOL], DT2,
                        kind="ExternalInput")
    wts = nc.dram_tensor("wts", [2, 2, 8, 128, 128], DT2, kind="ExternalInput")
    halfeye = nc.dram_tensor("halfeye", [128, 128], DT2, kind="ExternalInput")
    biases = nc.dram_tensor("biases", [128, 16], F32, kind="ExternalInput")
    if loop_R:
        dummyout = nc.dram_tensor("dummyout", [128, 4], F32,
                                  kind="ExternalOutput")
    else:
        out = nc.dram_tensor("out", [BL, Tn, D], F32, kind="ExternalOutput")

    NS = 2 * Tn + 3   # supersteps (L1 streams lag their L0 feed by 3)

    with tile.TileContext(nc) as tc:
        with (
            tc.tile_pool(name="wpool", bufs=1) as wpool,
            tc.tile_pool(name="stage", bufs=1) as stage,
            tc.tile_pool(name="xpool", bufs=8) as xpool,
            tc.tile_pool(name="hpool", bufs=8) as hpool,
            tc.tile_pool(name="cpool", bufs=1) as cpool,
            tc.tile_pool(name="gsb", bufs=6) as gsb,
            tc.tile_pool(name="tcp", bufs=4) as tcp,
            tc.tile_pool(name="ttp", bufs=4) as ttp,
            tc.tile_pool(name="ppl", bufs=4) as ppl,
            tc.tile_pool(name="outp", bufs=6) as outp,
            tc.tile_pool(name="dramp", bufs=1, space="DRAM") as dramp,
            tc.tile_pool(name="pif", bufs=2, space="PSUM") as pif,
            tc.tile_pool(name="pg", bufs=1, space="PSUM") as pg,
            tc.tile_pool(name="po", bufs=2, space="PSUM") as po,
            tc.tile_pool(name="pmg", bufs=1, space="PSUM") as pmg,
        ):
            # ---- constants / weights ----
            wslab = wpool.tile([128, 4096], DT2)
            for di in range(2):
                for l in range(2):
                    base = _wcol(di, l, 0)
                    nc.sync.dma_start(
                        wslab[:, base:base + 1024].rearrange(
                            "k (s m) -> k s m", s=8),
                        wts.ap()[di, l].rearrange("s k m -> k s m"))
            heye = wpool.tile([128, 128], DT2)
            nc.sync.dma_start(heye[:], halfeye.ap())
            bsb = wpool.tile([128, 16], F32)
            nc.sync.dma_start(bsb[:], biases.ap())

            if loop_R:
                out_int = dramp.tile([BL, Tn, D], F32)

                def out_ap():
                    return out_int[:]
            else:
                def out_ap():
                    return out.ap()

            z0 = wpool.tile([128, BCOL], DT2)
            nc.gpsimd.memset(z0[:], 0.0)

            # persistent cell states per direction: [c0|c1] pair tiles
            c_pair = {}
            for di in range(2):
                ct = cpool.tile([128, 2 * BCOL], DT2, tag=f"c{di}")
                nc.gpsimd.memset(ct[:], 0.0)
                c_pair[di] = ct

            # SBUF staging for layer-1 outputs awaiting merge
            stage_f = stage.tile([128, Th * BCOL], DT2, tag="stf")
            stage_b = stage.tile([128, Th * BCOL], DT2, tag="stb")

            h_prev = {(0, 0): z0[:], (0, 1): z0[:],
                      (1, 0): z0[:], (1, 1): z0[:]}
            xT_ready = {}
            mm_ctx = {}             # s -> (pif_t, pg_t, po_t)

            def x_load(s):
                m = s // 2
                if m >= Tn:
                    return
                di = s % 2
                tsrc = 0 if tiny_x else (m if di == 0 else Tn - 1 - m)
                with nc.named_scope("xload"):
                    xt = xpool.tile([128, BCOL], DT2)
                    nc.sync.dma_start(xt[:], xs.ap()[:, tsrc, :])
                xT_ready[s] = xt

            def stt(out_ap_, in0, in1, op1):
                # tensor_tensor runs in the 2x DVE mode for fp16 operands;
                # the 3-tensor scalar_tensor_tensor form would run at 1x.
                if op1 == ALU.add:
                    nc.vector.tensor_add(out_ap_, in0, in1)
                else:
                    nc.vector.tensor_mul(out_ap_, in0, in1)

            def stream_flags(s):
                """(st0, st1) for window s: stream0 = L0 of parity di,
                stream1 = L1 of the OPPOSITE direction, lagging 3 windows."""
                st0 = s // 2 < Tn
                if s % 2 == 1:
                    st1 = 3 <= s <= 2 * Tn + 1
                else:
                    st1 = 4 <= s <= 2 * Tn + 2
                return st0, st1

            def emit_mms(s):
                """Gate matmuls for window s. PE order: all input-side W
                matmuls first (rhs ready early: x, or the 3-window-old L0 h
                of the other parity), then the 4 recurrent U matmuls (the
                critical chain), then g and o last (their PSUM tiles are
                single-buffered and WAR-wait on this window's ACT reads)."""
                if s >= NS:
                    return
                di = s % 2
                st0, st1 = stream_flags(s)
                if not (st0 or st1):
                    return
                sc = nc.named_scope("gatemm"); sc.__enter__()
                pif_t = pif.tile([128, 1024], F32)
                pg_t = pg.tile([128, 512], F32)
                po_t = po.tile([128, 512], F32)
                # stream -> (input rhs, recurrent rhs, weight dir, layer)
                rhs = {}
                if st0:
                    rhs[0] = (xT_ready.pop(s)[:], h_prev[(di, 0)], di, 0)
                if st1:
                    rhs[1] = (h_prev[(1 - di, 0)], h_prev[(di, 1)],
                              1 - di, 1)

                def mm(dst, stream, slot, r, start, stop):
                    wd = rhs[stream][2]
                    nc.tensor.matmul(
                        dst, wslab[:, _wcol(wd, rhs[stream][3], slot):
                                   _wcol(wd, rhs[stream][3], slot) + 128],
                        r, start=start, stop=stop)

                # IMPORTANT hardware constraint (found empirically): a PSUM
                # region's W matmul and its accumulating U matmul must stay
                # close together in the PE stream — hoisting all W's ahead of
                # all U's silently corrupts the accumulation. Keep the proven
                # per-stream pattern [W_i, W_f][U_i, U_f] and per-region
                # [W, U] pairs for g and o.
                def ifdst(stream, gn):
                    base = 0 if gn == "i" else 512
                    return pif_t[:, base + stream * 256:
                                 base + stream * 256 + 256]

                for stream in rhs:
                    for gn in ("i", "f"):
                        mm(ifdst(stream, gn), stream, WSLOT[gn],
                           rhs[stream][0], True, True)
                    for gn in ("i", "f"):
                        mm(ifdst(stream, gn), stream, USLOT[gn],
                           rhs[stream][1], False, True)
                for stream in rhs:
                    dst = pg_t[:, stream * 256:stream * 256 + 256]
                    mm(dst, stream, WSLOT["g"], rhs[stream][0], True, True)
                    mm(dst, stream, USLOT["g"], rhs[stream][1], False, True)
                for stream in rhs:
                    dst = po_t[:, stream * 256:stream * 256 + 256]
                    mm(dst, stream, WSLOT["o"], rhs[stream][0], True, True)
                    mm(dst, stream, USLOT["o"], rhs[stream][1], False, True)
                sc.__exit__(None, None, None)
                mm_ctx[s] = (pif_t, pg_t, po_t)

            def finish_prev(pend):
                """tanh(c) + h-muls of window s-1 (ACT slot 3 + DVE)."""
                (di_p, st0, st1, tt1, gates_p, lo, hi, _po) = pend
                d1 = 1 - di_p          # stream1 direction
                c_t = c_pair[di_p]
                tc_t = tcp.tile([128, 2 * BCOL], DT2)
                with nc.named_scope("tanhc"):
                    nc.scalar.activation(tc_t[:, lo:hi], c_t[:, lo:hi],
                                         AF.Tanh)
                merge = None
                with nc.named_scope("hmul"):
                    if st0:
                        h_t = hpool.tile([128, BCOL], DT2, tag="h0")
                        stt(h_t[:], gates_p[:, 1024:1280], tc_t[:, 0:BCOL],
                            ALU.mult)
                        h_prev[(di_p, 0)] = h_t[:]
                    if st1:
                        o_ap = gates_p[:, 1280:1536]
                        stg = stage_f if d1 == 0 else stage_b
                        stage_this = (tt1 < Th) if d1 == 0 else (tt1 >= Th)
                        if stage_this:
                            soff = (tt1 if d1 == 0 else tt1 - Th) * BCOL
                            dst = stg[:, soff:soff + BCOL]
                            stt(dst, o_ap, tc_t[:, BCOL:2 * BCOL], ALU.mult)
                            h_prev[(di_p, 1)] = dst
                        else:
                            h_t = hpool.tile([128, BCOL], DT2, tag="h1")
                            stt(h_t[:], o_ap, tc_t[:, BCOL:2 * BCOL],
                                ALU.mult)
                            h_prev[(di_p, 1)] = h_t[:]
                            merge = (d1, tt1, h_t)
                return merge

            def bias_for(di, stream, gi):
                # stream0 = L0 of dir di; stream1 = L1 of dir 1-di
                d = di if stream == 0 else 1 - di
                return (bias_val if uniform_bias else
                        bsb[:, (d * 2 + stream) * 4 + gi:
                            (d * 2 + stream) * 4 + gi + 1])

            def emit_sig_if(s, st0, st1, gates, pif_t):
                """ACT slot 1: sigmoid over [i0|i1|f0|f1]."""
                di = s % 2
                if uniform_bias and st0 and st1:
                    with nc.named_scope("sigif"):
                        nc.scalar.activation(gates[:, 0:1024], pif_t[:],
                                             AF.Sigmoid, bias=bias_val)
                else:
                    for stream, active in ((0, st0), (1, st1)):
                        if not active:
                            continue
                        cc = stream * 256
                        nc.scalar.activation(gates[:, cc:cc + 256],
                                             pif_t[:, cc:cc + 256],
                                             AF.Sigmoid,
                                             bias=bias_for(di, stream, 0))
                        nc.scalar.activation(gates[:, 512 + cc:512 + cc + 256],
                                             pif_t[:, 512 + cc:512 + cc + 256],
                                             AF.Sigmoid,
                                             bias=bias_for(di, stream, 1))

            def emit_tanhg(s, st0, st1, gates, pg_t):
                """ACT slot 3: tanh over [g0|g1]."""
                di = s % 2
                if uniform_bias and st0 and st1:
                    with nc.named_scope("tanhg"):
                        nc.scalar.activation(gates[:, 1536:2048], pg_t[:],
                                             AF.Tanh, bias=bias_val)
                else:
                    for stream, active in ((0, st0), (1, st1)):
                        if not active:
                            continue
                        cc = stream * 256
                        nc.scalar.activation(gates[:, 1536 + cc:1536 + cc + 256],
                                             pg_t[:, cc:cc + 256],
                                             AF.Tanh,
                                             bias=bias_for(di, stream, 3))

            def emit_sigo(s, st0, st1, gates, po_t):
                """ACT slot 4: sigmoid over [o0|o1]."""
                di = s % 2
                if uniform_bias and st0 and st1:
                    with nc.named_scope("sigo"):
                        nc.scalar.activation(gates[:, 1024:1536], po_t[:],
                                             AF.Sigmoid, bias=bias_val)
                else:
                    for stream, active in ((0, st0), (1, st1)):
                        if not active:
                            continue
                        cc = stream * 256
                        nc.scalar.activation(gates[:, 1024 + cc:1024 + cc + 256],
                                             po_t[:, cc:cc + 256],
                                             AF.Sigmoid,
                                             bias=bias_for(di, stream, 2))

            def finish_merge(merge):
                """Layer-1 output merge on PE + GPSIMD copy + store."""
                di_p, tt1, h_t = merge
                sc = nc.named_scope("mergeout"); sc.__enter__()
                ostg = stage_b if di_p == 0 else stage_f
                ooff = (tt1 - Th if di_p == 0 else tt1) * BCOL
                other = ostg[:, ooff:ooff + BCOL]
                pm = pmg.tile([128, BCOL], F32)
                for j in range(NB):
                    mdst = pm[:, j * 128:j * 128 + 128]
                    nc.tensor.matmul(
                        mdst, h_t[:, j * 128:j * 128 + 128],
                        heye[:], start=True, stop=False)
                    nc.tensor.matmul(
                        mdst, other[:, j * 128:j * 128 + 128],
                        heye[:], start=False, stop=True)
                ost = outp.tile([128, BCOL], F32)
                nc.vector.tensor_copy(ost[:], pm[:])
                nc.sync.dma_start(
                    out_ap()[:, tt1, :].rearrange("(j p) d -> p j d", j=NB),
                    ost[:].rearrange("p (j d) -> p j d", j=NB))
                sc.__exit__(None, None, None)

            import contextlib
            pending = None          # state of superstep s-1
            if loop_R:
                cnt = wpool.tile([128, 4], F32, tag="cnt")
                nc.gpsimd.memset(cnt[:], 0.0)
            loop_cm = tc.For_i(0, loop_R, 1) if loop_R else \
                contextlib.nullcontext()
            with loop_cm:
             if loop_R:
                 nc.vector.tensor_scalar_add(cnt[:], cnt[:], 1.0)
             for rep in range(reps):
              if rep > 0 or loop_R:
                for di_ in range(2):
                    nc.gpsimd.memset(c_pair[di_][:], 0.0)
                h_prev.update({(0, 0): z0[:], (0, 1): z0[:],
                               (1, 0): z0[:], (1, 1): z0[:]})
                pending = None
              for s in range(NS):
                di = s % 2
                st0, st1 = stream_flags(s)
                # stream1 output timestep: odd window hosts L1f at t=(s-3)/2,
                # even window hosts L1b at reversed index (s-4)/2
                if di == 1:
                    tt1 = (s - 3) // 2
                else:
                    tt1 = Tn - 1 - (s - 4) // 2
                lo = 0 if st0 else BCOL
                hi = 2 * BCOL if st1 else BCOL

                # ---- prologue / x prefetch ----
                if s == 0:
                    for ps in range(4):
                        x_load(ps)
                    emit_mms(0)
                if s + 4 < NS:
                    x_load(s + 4)

                pif_t, pg_t, po_t = mm_ctx.pop(s)
                gates = gsb.tile([128, 2048], DT2)
                c_t = c_pair[di]

                # ---- ACT slot 1: sigo(s-1), deferred a full window: its
                # o-psum has been ready since window s-2, so this slot never
                # stalls, and it pads the deadline of the U-matmul chain
                # feeding slot 2 by 612ns ----
                if pending is not None:
                    (_dp, p_st0, p_st1, _tt, p_gates, _lo, _hi,
                     p_po) = pending
                    emit_sigo(s - 1, p_st0, p_st1, p_gates, p_po)

                # ---- ACT slot 2: sig_if(s) ----
                emit_sig_if(s, st0, st1, gates, pif_t)

                # ---- DVE: tt(s) = f * c (before the h-muls in DVE order,
                # it only needs sig_if) ----
                tt_t = ttp.tile([128, 2 * BCOL], DT2)
                with nc.named_scope("ttmul"):
                    stt(tt_t[:, lo:hi], gates[:, 512 + lo:512 + hi],
                        c_t[:, lo:hi], ALU.mult)

                # ---- ACT slot 3 + DVE h-muls: finish superstep s-1 ----
                merge = None
                if pending is not None:
                    merge = finish_prev(pending)
                    pending = None

                # ---- ACT slot 4: tanhg(s) ----
                emit_tanhg(s, st0, st1, gates, pg_t)

                # ---- DVE: p(s) = i * g ; c(s) = tt + p ----
                sc = nc.named_scope("combine"); sc.__enter__()
                p_t = ppl.tile([128, 2 * BCOL], DT2)
                stt(p_t[:, lo:hi], gates[:, lo:hi],
                    gates[:, 1536 + lo:1536 + hi], ALU.mult)
                stt(c_t[:, lo:hi], tt_t[:, lo:hi], p_t[:, lo:hi], ALU.add)
                sc.__exit__(None, None, None)

                # ---- PE: gate matmuls for superstep s+1 ----
                emit_mms(s + 1)

                # ---- PE merge + DVE copy + store of superstep s-1 ----
                if merge is not None:
                    finish_merge(merge)

                pending = (di, st0, st1, tt1, gates, lo, hi, po_t)

              if pending is not None:
                (_dp, p_st0, p_st1, _tt, p_gates, _lo, _hi, p_po) = pending
                emit_sigo(NS - 1, p_st0, p_st1, p_gates, p_po)
                merge = finish_prev(pending)
                if merge is not None:
                    finish_merge(merge)
                pending = None
             if loop_R:
                nc.sync.dma_start(dummyout.ap(), cnt[:])
    nc.compile()
    return nc


def _prep_weights(Wf, Uf, Wb, Ub):
    wts = np.zeros((2, 2, 8, 128, 128), dtype=DT2_NP)
    for di, (Wd, Ud) in enumerate(((Wf, Uf), (Wb, Ub))):
        for l in range(2):
            for si, sname in enumerate(SLOTS):
                mat = Wd[l] if sname[0] == "W" else Ud[l]
                g = GATE_COLS[sname[1]]
                wts[di, l, si] = np.asarray(
                    mat[:, g:g + 128], dtype=np.float32).astype(DT2_NP)
    return wts


def _prep_aux(bf, bb):
    halfeye = (0.5 * np.eye(128, dtype=np.float32)).astype(DT2_NP)
    biases = np.zeros((128, 16), dtype=np.float32)
    for di, bd in enumerate((bf, bb)):
        for l in range(2):
            for gi, gname in enumerate(("i", "f", "o", "g")):
                g = GATE_COLS[gname]
                biases[:, (di * 2 + l) * 4 + gi] = bd[l, g:g + 128]
    return halfeye, biases


def _prep_x(x):
    """Per-core [D, T, BL] fp16 transposes of the batch shards."""
    x16 = np.asarray(x, dtype=np.float32).astype(DT2_NP)
    return [np.ascontiguousarray(
        x16[c * BL:(c + 1) * BL].transpose(2, 1, 0))
        for c in range(NCORES)]


def kernel(x, Wf, Uf, bf, Wb, Ub, bb):
    bf = np.asarray(bf, dtype=np.float32)
    bb = np.asarray(bb, dtype=np.float32)

    bval = float(bf.flat[0])
    uniform = bool(np.all(bf == bval) and np.all(bb == bval))

    key = (uniform, bval if uniform else None)
    if key not in _cache:
        _cache[key] = _build(uniform, bval if uniform else 0.0)
    nc = _cache[key]

    wts = _prep_weights(Wf, Uf, Wb, Ub)
    halfeye, biases = _prep_aux(bf, bb)
    xcores = _prep_x(x)

    in_maps = []
    for c in range(NCORES):
        in_maps.append({
            "xs": xcores[c],
            "wts": wts,
            "halfeye": halfeye,
            "biases": biases,
        })
    res = run_bass_kernel_spmd(nc, in_maps, core_ids=list(range(NCORES)))
    return np.concatenate([res.results[c]["out"] for c in range(NCORES)],
                          axis=0).astype(np.float32)


# revision 30
# speedup vs baseline: 1.2094x; 1.2094x over previous
"""Trainium2 Bass kernel for nn_BiLSTM_57440892617018.

2-layer bidirectional LSTM (independent fw / bw stacks, merge_mode='ave'),
B=2048, T=200, D=U=128. Data-parallel over batch across 8 NeuronCores.

Per-core structure: ONE software-pipelined loop, 2 recurrent streams per
superstep (even = fw pair L0f/L1f with L1 lagging one timestep, odd = bw
pair on the time-reversed sequence). The schedule is built around keeping
the ACT engine (the busy-time bottleneck: 5 activations x 256 batch cols
per stream-timestep at ~0.83ns/col) 100% busy:

  ACT order per superstep s: [tanh_c(s-1) | sig_if(s) | tanh_g(s) | sig_o(s)]

tanh_c of the previous superstep runs while THIS superstep's DVE combine
(c = f*c + i*g) finishes, so ACT never waits on DVE. All DVE elementwise
work is emitted as scalar_tensor_tensor (4x DVE mode for fp16 SBUF
operands). x is pre-transposed on the host to [D, T, B] fp16 so DMA loads
land directly as matmul-ready [d, b] tiles (no PE transposes, no DVE
copies). PE runs fp16 matmuls: per superstep 16 gate matmuls (x/W first,
then h/U, g and o last for PSUM WAR friendliness) plus the layer-1 output
merge (0.5*I matmuls, which also produce the [b, d] layout the output DMA
needs). The PSUM->SBUF output copy runs on the otherwise-idle GPSIMD
engine.
"""

import numpy as np
import ml_dtypes

import concourse.bass as bass
import concourse.tile as tile
from concourse import bacc, mybir
from concourse.bass_utils import run_bass_kernel_spmd

F32 = mybir.dt.float32
DT2 = mybir.dt.float16
DT2_NP = np.float16
AF = mybir.ActivationFunctionType
ALU = mybir.AluOpType

B, T, D, U = 2048, 200, 128, 128
NCORES = 8
BL = B // NCORES          # 256 batch per core
NB = BL // 128            # 2 b-tiles
BCOL = 128 * NB           # 256 free columns (batch)

# gate order inside the reference 4u axis: i, f, g, o
GATE_COLS = {"i": 0, "f": 128, "g": 256, "o": 384}
SLOTS = ["Wi", "Wf", "Wo", "Wg", "Ui", "Uf", "Uo", "Ug"]
# W slot per gate / U slot per gate (indices into SLOTS)
WSLOT = {"i": 0, "f": 1, "g": 3, "o": 2}
USLOT = {"i": 4, "f": 5, "g": 7, "o": 6}

_cache = {}


def _wcol(di, l, slot):
    return ((di * 2 + l) * 8 + slot) * 128


def _build(uniform_bias, bias_val, Tn=T, reps=1, loop_R=0, tiny_x=False):
    nc = bacc.Bacc("TRN2", target_bir_lowering=False, debug=False,
                   num_devices=NCORES)
    Th = Tn // 2

    # x pre-transposed on host: [D, T, B] fp16 (contiguous b per (d, t))
    xs = nc.dram_tensor("xs", [D, 1 if tiny_x else Tn, BCOL], DT2,
                        kind="ExternalInput")
    wts = nc.dram_tensor("wts", [2, 2, 8, 128, 128], DT2, kind="ExternalInput")
    halfeye = nc.dram_tensor("halfeye", [128, 128], DT2, kind="ExternalInput")
    biases = nc.dram_tensor("biases", [128, 16], F32, kind="ExternalInput")
    if loop_R:
        dummyout = nc.dram_tensor("dummyout", [128, 4], F32,
                                  kind="ExternalOutput")
    else:
        out = nc.dram_tensor("out", [BL, Tn, D], F32, kind="ExternalOutput")

    NS = 2 * Tn + 3   # supersteps (L1 streams lag their L0 feed by 3)

    with tile.TileContext(nc) as tc:
        with (
            tc.tile_pool(name="wpool", bufs=1) as wpool,
            tc.tile_pool(name="stage", bufs=1) as stage,
            tc.tile_pool(name="xpool", bufs=8) as xpool,
            tc.tile_pool(name="hpool", bufs=8) as hpool,
            tc.tile_pool(name="cpool", bufs=1) as cpool,
            tc.tile_pool(name="gsb", bufs=6) as gsb,
            tc.tile_pool(name="tcp", bufs=4) as tcp,
            tc.tile_pool(name="ttp", bufs=4) as ttp,
            tc.tile_pool(name="ppl", bufs=4) as ppl,
            tc.tile_pool(name="outp", bufs=6) as outp,
            tc.tile_pool(name="dramp", bufs=1, space="DRAM") as dramp,
            tc.tile_pool(name="pif", bufs=2, space="PSUM") as pif,
            tc.tile_pool(name="pg", bufs=1, space="PSUM") as pg,
            tc.tile_pool(name="po", bufs=2, space="PSUM") as po,
            tc.tile_pool(name="pmg", bufs=1, space="PSUM") as pmg,
        ):
            # ---- constants / weights ----
            wslab = wpool.tile([128, 4096], DT2)
            for di in range(2):
                for l in range(2):
                    base = _wcol(di, l, 0)
                    nc.sync.dma_start(
                        wslab[:, base:base + 1024].rearrange(
                            "k (s m) -> k s m", s=8),
                        wts.ap()[di, l].rearrange("s k m -> k s m"))
            heye = wpool.tile([128, 128], DT2)
            nc.sync.dma_start(heye[:], halfeye.ap())
            bsb = wpool.tile([128, 16], F32)
            nc.sync.dma_start(bsb[:], biases.ap())

            if loop_R:
                out_int = dramp.tile([BL, Tn, D], F32)

                def out_ap():
                    return out_int[:]
            else:
                def out_ap():
                    return out.ap()

            z0 = wpool.tile([128, BCOL], DT2)
            nc.gpsimd.memset(z0[:], 0.0)

            # persistent cell states per direction: [c0|c1] pair tiles
            c_pair = {}
            for di in range(2):
                ct = cpool.tile([128, 2 * BCOL], DT2, tag=f"c{di}")
                nc.gpsimd.memset(ct[:], 0.0)
                c_pair[di] = ct

            # SBUF staging for layer-1 outputs awaiting merge
            stage_f = stage.tile([128, Th * BCOL], DT2, tag="stf")
            stage_b = stage.tile([128, Th * BCOL], DT2, tag="stb")

            h_prev = {(0, 0): z0[:], (0, 1): z0[:],
                      (1, 0): z0[:], (1, 1): z0[:]}
            xT_ready = {}
            mm_ctx = {}             # s -> (pif_t, pg_t, po_t)

            def x_load(s):
                m = s // 2
                if m >= Tn:
                    return
                di = s % 2
                tsrc = 0 if tiny_x else (m if di == 0 else Tn - 1 - m)
                with nc.named_scope("xload"):
                    xt = xpool.tile([128, BCOL], DT2)
                    nc.sync.dma_start(xt[:], xs.ap()[:, tsrc, :])
                xT_ready[s] = xt

            def stt(out_ap_, in0, in1, op1):
                # tensor_tensor runs in the 2x DVE mode for fp16 operands;
                # the 3-tensor scalar_tensor_tensor form would run at 1x.
                if op1 == ALU.add:
                    nc.vector.tensor_add(out_ap_, in0, in1)
                else:
                    nc.vector.tensor_mul(out_ap_, in0, in1)

            def stream_flags(s):
                """(st0, st1) for window s: stream0 = L0 of parity di,
                stream1 = L1 of the OPPOSITE direction, lagging 3 windows."""
                st0 = s // 2 < Tn
                if s % 2 == 1:
                    st1 = 3 <= s <= 2 * Tn + 1
                else:
                    st1 = 4 <= s <= 2 * Tn + 2
                return st0, st1

            def emit_mms(s):
                """Gate matmuls for window s. PE order: all input-side W
                matmuls first (rhs ready early: x, or the 3-window-old L0 h
                of the other parity), then the 4 recurrent U matmuls (the
                critical chain), then g and o last (their PSUM tiles are
                single-buffered and WAR-wait on this window's ACT reads)."""
                if s >= NS:
                    return
                di = s % 2
                st0, st1 = stream_flags(s)
                if not (st0 or st1):
                    return
                sc = nc.named_scope("gatemm"); sc.__enter__()
                pif_t = pif.tile([128, 1024], F32)
                pg_t = pg.tile([128, 512], F32)
                po_t = po.tile([128, 512], F32)
                # stream -> (input rhs, recurrent rhs, weight dir, layer)
                rhs = {}
                if st0:
                    rhs[0] = (xT_ready.pop(s)[:], h_prev[(di, 0)], di, 0)
                if st1:
                    rhs[1] = (h_prev[(1 - di, 0)], h_prev[(di, 1)],
                              1 - di, 1)

                def mm(dst, stream, slot, r, start, stop):
                    wd = rhs[stream][2]
                    nc.tensor.matmul(
                        dst, wslab[:, _wcol(wd, rhs[stream][3], slot):
                                   _wcol(wd, rhs[stream][3], slot) + 128],
                        r, start=start, stop=stop)

                # IMPORTANT hardware constraint (found empirically): a PSUM
                # region's W matmul and its accumulating U matmul must stay
                # close together in the PE stream — hoisting all W's ahead of
                # all U's silently corrupts the accumulation. Keep the proven
                # per-stream pattern [W_i, W_f][U_i, U_f] and per-region
                # [W, U] pairs for g and o.
                def ifdst(stream, gn):
                    base = 0 if gn == "i" else 512
                    return pif_t[:, base + stream * 256:
                                 base + stream * 256 + 256]

                for stream in rhs:
                    for gn in ("i", "f"):
                        mm(ifdst(stream, gn), stream, WSLOT[gn],
                           rhs[stream][0], True, False)
                    for gn in ("i", "f"):
                        mm(ifdst(stream, gn), stream, USLOT[gn],
                           rhs[stream][1], False, True)
                for stream in rhs:
                    dst = pg_t[:, stream * 256:stream * 256 + 256]
                    mm(dst, stream, WSLOT["g"], rhs[stream][0], True, False)
                    mm(dst, stream, USLOT["g"], rhs[stream][1], False, True)
                for stream in rhs:
                    dst = po_t[:, stream * 256:stream * 256 + 256]
                    mm(dst, stream, WSLOT["o"], rhs[stream][0], True, False)
                    mm(dst, stream, USLOT["o"], rhs[stream][1], False, True)
                sc.__exit__(None, None, None)
                mm_ctx[s] = (pif_t, pg_t, po_t)

            def finish_prev(pend):
                """tanh(c) + h-muls of window s-1 (ACT slot 3 + DVE)."""
                (di_p, st0, st1, tt1, gates_p, lo, hi, _po) = pend
                d1 = 1 - di_p          # stream1 direction
                c_t = c_pair[di_p]
                tc_t = tcp.tile([128, 2 * BCOL], DT2)
                with nc.named_scope("tanhc"):
                    nc.scalar.activation(tc_t[:, lo:hi], c_t[:, lo:hi],
                                         AF.Tanh)
                merge = None
                with nc.named_scope("hmul"):
                    if st0:
                        h_t = hpool.tile([128, BCOL], DT2, tag="h0")
                        stt(h_t[:], gates_p[:, 1024:1280], tc_t[:, 0:BCOL],
                            ALU.mult)
                        h_prev[(di_p, 0)] = h_t[:]
                    if st1:
                        o_ap = gates_p[:, 1280:1536]
                        stg = stage_f if d1 == 0 else stage_b
                        stage_this = (tt1 < Th) if d1 == 0 else (tt1 >= Th)
                        if stage_this:
                            soff = (tt1 if d1 == 0 else tt1 - Th) * BCOL
                            dst = stg[:, soff:soff + BCOL]
                            stt(dst, o_ap, tc_t[:, BCOL:2 * BCOL], ALU.mult)
                            h_prev[(di_p, 1)] = dst
                        else:
                            h_t = hpool.tile([128, BCOL], DT2, tag="h1")
                            stt(h_t[:], o_ap, tc_t[:, BCOL:2 * BCOL],
                                ALU.mult)
                            h_prev[(di_p, 1)] = h_t[:]
                            merge = (d1, tt1, h_t)
                return merge

            def bias_for(di, stream, gi):
                # stream0 = L0 of dir di; stream1 = L1 of dir 1-di
                d = di if stream == 0 else 1 - di
                return (bias_val if uniform_bias else
                        bsb[:, (d * 2 + stream) * 4 + gi:
                            (d * 2 + stream) * 4 + gi + 1])

            def emit_sig_if(s, st0, st1, gates, pif_t):
                """ACT slot 1: sigmoid over [i0|i1|f0|f1]."""
                di = s % 2
                if uniform_bias and st0 and st1:
                    with nc.named_scope("sigif"):
                        nc.scalar.activation(gates[:, 0:1024], pif_t[:],
                                             AF.Sigmoid, bias=bias_val)
                else:
                    for stream, active in ((0, st0), (1, st1)):
                        if not active:
                            continue
                        cc = stream * 256
                        nc.scalar.activation(gates[:, cc:cc + 256],
                                             pif_t[:, cc:cc + 256],
                                             AF.Sigmoid,
                                             bias=bias_for(di, stream, 0))
                        nc.scalar.activation(gates[:, 512 + cc:512 + cc + 256],
                                             pif_t[:, 512 + cc:512 + cc + 256],
                                             AF.Sigmoid,
                                             bias=bias_for(di, stream, 1))

            def emit_tanhg(s, st0, st1, gates, pg_t):
                """ACT slot 3: tanh over [g0|g1]."""
                di = s % 2
                if uniform_bias and st0 and st1:
                    with nc.named_scope("tanhg"):
                        nc.scalar.activation(gates[:, 1536:2048], pg_t[:],
                                             AF.Tanh, bias=bias_val)
                else:
                    for stream, active in ((0, st0), (1, st1)):
                        if not active:
                            continue
                        cc = stream * 256
                        nc.scalar.activation(gates[:, 1536 + cc:1536 + cc + 256],
                                             pg_t[:, cc:cc + 256],
                                             AF.Tanh,
                                             bias=bias_for(di, stream, 3))

            def emit_sigo(s, st0, st1, gates, po_t):
                """ACT slot 4: sigmoid over [o0|o1]."""
                di = s % 2
                if uniform_bias and st0 and st1:
                    with nc.named_scope("sigo"):
                        nc.scalar.activation(gates[:, 1024:1536], po_t[:],
                                             AF.Sigmoid, bias=bias_val)
                else:
                    for stream, active in ((0, st0), (1, st1)):
                        if not active:
                            continue
                        cc = stream * 256
                        nc.scalar.activation(gates[:, 1024 + cc:1024 + cc + 256],
                                             po_t[:, cc:cc + 256],
                                             AF.Sigmoid,
                                             bias=bias_for(di, stream, 2))

            def finish_merge(merge):
                """Layer-1 output merge on PE + GPSIMD copy + store."""
                di_p, tt1, h_t = merge
                sc = nc.named_scope("mergeout"); sc.__enter__()
                ostg = stage_b if di_p == 0 else stage_f
                ooff = (tt1 - Th if di_p == 0 else tt1) * BCOL
                other = ostg[:, ooff:ooff + BCOL]
                pm = pmg.tile([128, BCOL], F32)
                for j in range(NB):
                    mdst = pm[:, j * 128:j * 128 + 128]
                    nc.tensor.matmul(
                        mdst, h_t[:, j * 128:j * 128 + 128],
                        heye[:], start=True, stop=False)
                    nc.tensor.matmul(
                        mdst, other[:, j * 128:j * 128 + 128],
                        heye[:], start=False, stop=True)
                ost = outp.tile([128, BCOL], F32)
                nc.vector.tensor_copy(ost[:], pm[:])
                nc.sync.dma_start(
                    out_ap()[:, tt1, :].rearrange("(j p) d -> p j d", j=NB),
                    ost[:].rearrange("p (j d) -> p j d", j=NB))
                sc.__exit__(None, None, None)

            import contextlib
            pending = None          # state of superstep s-1
            if loop_R:
                cnt = wpool.tile([128, 4], F32, tag="cnt")
                nc.gpsimd.memset(cnt[:], 0.0)
            loop_cm = tc.For_i(0, loop_R, 1) if loop_R else \
                contextlib.nullcontext()
            with loop_cm:
             if loop_R:
                 nc.vector.tensor_scalar_add(cnt[:], cnt[:], 1.0)
             for rep in range(reps):
              if rep > 0 or loop_R:
                for di_ in range(2):
                    nc.gpsimd.memset(c_pair[di_][:], 0.0)
                h_prev.update({(0, 0): z0[:], (0, 1): z0[:],
                               (1, 0): z0[:], (1, 1): z0[:]})
                pending = None
              for s in range(NS):
                di = s % 2
                st0, st1 = stream_flags(s)
                # stream1 output timestep: odd window hosts L1f at t=(s-3)/2,
                # even window hosts L1b at reversed index (s-4)/2
                if di == 1:
                    tt1 = (s - 3) // 2
                else:
                    tt1 = Tn - 1 - (s - 4) // 2
                lo = 0 if st0 else BCOL
                hi = 2 * BCOL if st1 else BCOL

                # ---- prologue / x prefetch ----
                if s == 0:
                    for ps in range(4):
                        x_load(ps)
                    emit_mms(0)
                if s + 4 < NS:
                    x_load(s + 4)

                pif_t, pg_t, po_t = mm_ctx.pop(s)
                gates = gsb.tile([128, 2048], DT2)
                c_t = c_pair[di]

                # ---- ACT slot 1: sigo(s-1), deferred a full window: its
                # o-psum has been ready since window s-2, so this slot never
                # stalls, and it pads the deadline of the U-matmul chain
                # feeding slot 2 by 612ns ----
                if pending is not None:
                    (_dp, p_st0, p_st1, _tt, p_gates, _lo, _hi,
                     p_po) = pending
                    emit_sigo(s - 1, p_st0, p_st1, p_gates, p_po)

                # ---- ACT slot 2: sig_if(s) ----
                emit_sig_if(s, st0, st1, gates, pif_t)

                # ---- DVE: tt(s) = f * c (before the h-muls in DVE order,
                # it only needs sig_if) ----
                tt_t = ttp.tile([128, 2 * BCOL], DT2)
                with nc.named_scope("ttmul"):
                    stt(tt_t[:, lo:hi], gates[:, 512 + lo:512 + hi],
                        c_t[:, lo:hi], ALU.mult)

                # ---- ACT slot 3 + DVE h-muls: finish superstep s-1 ----
                merge = None
                if pending is not None:
                    merge = finish_prev(pending)
                    pending = None

                # ---- ACT slot 4: tanhg(s) ----
                emit_tanhg(s, st0, st1, gates, pg_t)

                # ---- DVE: p(s) = i * g ; c(s) = tt + p ----
                sc = nc.named_scope("combine"); sc.__enter__()
                p_t = ppl.tile([128, 2 * BCOL], DT2)
                stt(p_t[:, lo:hi], gates[:, lo:hi],
                    gates[:, 1536 + lo:1536 + hi], ALU.mult)
                stt(c_t[:, lo:hi], tt_t[:, lo:hi], p_t[:, lo:hi], ALU.add)
                sc.__exit__(None, None, None)

                # ---- PE: gate matmuls for superstep s+1 ----
                emit_mms(s + 1)

                # ---- PE merge + DVE copy + store of superstep s-1 ----
                if merge is not None:
                    finish_merge(merge)

                pending = (di, st0, st1, tt1, gates, lo, hi, po_t)

              if pending is not None:
                (_dp, p_st0, p_st1, _tt, p_gates, _lo, _hi, p_po) = pending
                emit_sigo(NS - 1, p_st0, p_st1, p_gates, p_po)
                merge = finish_prev(pending)
                if merge is not None:
                    finish_merge(merge)
                pending = None
             if loop_R:
                nc.sync.dma_start(dummyout.ap(), cnt[:])
    nc.compile()
    return nc


def _prep_weights(Wf, Uf, Wb, Ub):
    wts = np.zeros((2, 2, 8, 128, 128), dtype=DT2_NP)
    for di, (Wd, Ud) in enumerate(((Wf, Uf), (Wb, Ub))):
        for l in range(2):
            for si, sname in enumerate(SLOTS):
                mat = Wd[l] if sname[0] == "W" else Ud[l]
                g = GATE_COLS[sname[1]]
                wts[di, l, si] = np.asarray(
                    mat[:, g:g + 128], dtype=np.float32).astype(DT2_NP)
    return wts


def _prep_aux(bf, bb):
    halfeye = (0.5 * np.eye(128, dtype=np.float32)).astype(DT2_NP)
    biases = np.zeros((128, 16), dtype=np.float32)
    for di, bd in enumerate((bf, bb)):
        for l in range(2):
            for gi, gname in enumerate(("i", "f", "o", "g")):
                g = GATE_COLS[gname]
                biases[:, (di * 2 + l) * 4 + gi] = bd[l, g:g + 128]
    return halfeye, biases


def _prep_x(x):
    """Per-core [D, T, BL] fp16 transposes of the batch shards."""
    x16 = np.asarray(x, dtype=np.float32).astype(DT2_NP)
    return [np.ascontiguousarray(
        x16[c * BL:(c + 1) * BL].transpose(2, 1, 0))
        for c in range(NCORES)]


def kernel(x, Wf, Uf, bf, Wb, Ub, bb):
    bf = np.asarray(bf, dtype=np.float32)
    bb = np.asarray(bb, dtype=np.float32)

    bval = float(bf.flat[0])
    uniform = bool(np.all(bf == bval) and np.all(bb == bval))

    key = (uniform, bval if uniform else None)
    if key not in _cache:
        _cache[key] = _build(uniform, bval if uniform else 0.0)
    nc = _cache[key]

    wts = _prep_weights(Wf, Uf, Wb, Ub)
    halfeye, biases = _prep_aux(bf, bb)
    xcores = _prep_x(x)

    in_maps = []
    for c in range(NCORES):
        in_maps.append({
            "xs": xcores[c],
            "wts": wts,
            "halfeye": halfeye,
            "biases": biases,
        })
    res = run_bass_kernel_spmd(nc, in_maps, core_ids=list(range(NCORES)))
    return np.concatenate([res.results[c]["out"] for c in range(NCORES)],
                          axis=0).astype(np.float32)
